# revision 21
# baseline (speedup 1.0000x reference)
"""Trainium2 Bass kernel for nn_MultiLatentAttention (B=2, S=2048, E=1024, H=16, P=64).

Math (exact reassociation of the reference):
  q = (x@WQ)@proj_w + proj_b          ->  x @ (WQ@proj_w) + proj_b
  attn1 - lam*attn2                   ->  q' @ k^T with q' = [s*q1, -s*lam*q2]
  (q'k^T) v                           ->  q' @ (k^T v)      (linear attention, no softmax)
  heads @ result_weight               ->  base @ W_eff,  W_eff[p,e] = sum_h (h+1)*RW[h*64+p, e]

Sharding: 8 cores, token-parallel for q/base/out (512 tokens each).  k^T v needs a
full-batch reduction; collectives cost 50-80us on this runtime, so each core instead
computes k,v over its ENTIRE batch (x^T for the full batch is staged per-core, fp16,
with columns rotated so the core's own q-tokens are columns 0:512 -- k^T v is
permutation-invariant over tokens).
"""

import math

import numpy as np

import concourse.bass as bass
import concourse.tile as tile
from concourse import mybir
from concourse.bass_utils import run_bass_kernel_spmd

E = 1024
H = 16
P = 64        # per-head width (latent/H)
B = 2
S = 2048
N_CORES = 8
SH = 512      # q-tokens per core
KO = E // 128    # 8 contraction chunks
CH = 8           # xt DMA chunks (256 tokens each)
TPC = S // CH    # 256 tokens per DMA chunk
SUB = S // 128   # 16 compute sub-chunks of 128 tokens

F16 = mybir.dt.float16
F32 = mybir.dt.float32


def _fix_excess_waits(nc, keep=1):
    """Split instructions with >keep sem waits (this walrus rejects multi-wait Drains)."""
    n_fixed = 0
    for f in nc.m.functions:
        for bb in f.blocks:
            insts = bb.instructions
            i = 0
            while i < len(insts):
                inst = insts[i]
                si = inst.sync_info
                waits = list(si.on_wait) if si is not None else []
                if len(waits) > keep:
                    excess, kept = waits[:-keep], waits[-keep:]
                    inst.sync_info = mybir.SyncInfo(on_wait=kept, on_update=list(si.on_update))
                    for k, w in enumerate(excess):
                        ev = mybir.InstEventSemaphore(
                            name=nc.get_next_instruction_name(),
                            engine=inst.engine, ins=[], outs=[],
                            sync_info=mybir.SyncInfo(on_wait=[w], on_update=[]),
                        )
                        nc.register_instruction(ev)
                        insts.insert(i + k, ev)
                    i += len(excess)
                    n_fixed += 1
                i += 1
    return n_fixed


def build_bass():
    nc = bass.Bass(num_devices=N_CORES, enable_partition_id=False)
    # xt: [128(ki), CH, KO, TPC] -- per-partition contiguous per chunk
    xt = nc.declare_dram_parameter("xt", [128, CH, KO, TPC], F16, isOutput=False)
    # wcomb: [wkvq (KO*192) | rows(448, partition 0 only)] per partition
    WCOLS = KO * 3 * P + 448
    wcomb = nc.declare_dram_parameter("wcomb", [128, WCOLS], F16, isOutput=False)
    weff = nc.declare_dram_parameter("weff", [P, E + 1], F16, isOutput=False)
    out = nc.declare_dram_parameter("out", [SH, E], F32, isOutput=True)

    with tile.TileContext(nc) as tc:
        with (
            tc.tile_pool(name="singles", bufs=1) as singles,
            tc.tile_pool(name="xtp", bufs=CH) as xtp,
            tc.tile_pool(name="kvp", bufs=1) as kvp,
            tc.tile_pool(name="small", bufs=1) as small,
            tc.tile_pool(name="outp", bufs=3) as outp,
            tc.tile_pool(name="pskv", bufs=3, space="PSUM") as pskv,
            tc.tile_pool(name="psacc", bufs=1, space="PSUM") as psacc,
            tc.tile_pool(name="pso", bufs=3, space="PSUM") as pso,
        ):
            # ---- first xt chunk, then weights, then the rest of xt ----
            xt_tiles = [None] * CH

            def load_chunk(i, split=False):
                t = xtp.tile([128, KO, TPC], F16, tag="xt")
                if split:
                    nc.sync.dma_start(out=t[:, :, 0:128], in_=xt[:, i, :, 0:128])
                    nc.sync.dma_start(out=t[:, :, 128:TPC], in_=xt[:, i, :, 128:TPC])
                else:
                    nc.sync.dma_start(out=t, in_=xt[:, i])
                xt_tiles[i] = t

            wc_sb = singles.tile([128, KO * 3 * P + 448], F16)
            nc.sync.dma_start(out=wc_sb[:, 0:2 * 3 * P],
                              in_=wcomb[:, 0:2 * 3 * P])
            load_chunk(0, split=True)
            nc.sync.dma_start(out=wc_sb[:, 2 * 3 * P:],
                              in_=wcomb[:, 2 * 3 * P:])
            wkvq_sb = wc_sb[:, 0:KO * 3 * P].rearrange("p (ko c) -> p ko c", ko=KO)
            rows_sb = wc_sb[0:1, KO * 3 * P:]
            bkv_sb = rows_sb[:, 0:128]
            bq_sb = rows_sb[:, 128:192]
            ones_sb = rows_sb[:, 192:448]
            for i in range(1, CH):
                load_chunk(i)
            weff_sb = singles.tile([P, E + 1], F16)
            nc.sync.dma_start(out=weff_sb, in_=weff[:, :])

            bias_bc = singles.tile([128, 2 * P], F16)
            nc.sync.dma_start(
                out=bias_bc,
                in_=bass.AP(tensor=wcomb.ap().tensor, offset=KO * 3 * P,
                            ap=[[0, 128], [1, 2 * P]]))

            kv_sb = kvp.tile([128, SUB, 2 * P], F16)

            # ---- k|v for every token sub-chunk (dense PE stream) ----
            def kv_chunk(j):
                i, half = j // 2, (j % 2) * 128
                ps = pskv.tile([128, 2 * P], F32, tag="kv")
                for ko in range(KO):
                    nc.tensor.matmul(ps, xt_tiles[i][:, ko, half:half + 128],
                                     wkvq_sb[:, ko, 0:2 * P],
                                     start=(ko == 0), stop=False)
                nc.tensor.matmul(ps, ones_sb[:, 0:128], bkv_sb, start=False, stop=True)
                if j % 2 == 0:
                    nc.vector.tensor_copy(out=kv_sb[:, j], in_=ps)
                else:
                    nc.scalar.copy(out=kv_sb[:, j], in_=ps)

            for j in range(4):
                kv_chunk(j)

            # ---- qT = wq^T @ xt[:, 0:512] + bq  -> [P, SH] (ko-outer, N=256) ----
            ps_q = psacc.tile([P, SH], F32, tag="q")
            for i in range(2):
                for ko in range(KO):
                    nc.tensor.matmul(ps_q[:, i * TPC:(i + 1) * TPC],
                                     wkvq_sb[:, ko, 2 * P:3 * P],
                                     xt_tiles[i][:, ko],
                                     start=(ko == 0), stop=(ko == KO - 1),
                                     skip_group_check=True)
            qT_sb = small.tile([P, SH], F16)
            nc.scalar.activation(out=qT_sb, in_=ps_q,
                                 func=mybir.ActivationFunctionType.Identity,
                                 bias=weff_sb[:, E:E + 1])

            # ---- M = k^T v, accumulated in groups of 4 sub-chunks ----
            ps_m = psacc.tile([P, P], F32, tag="m")

            def m_group(j0):
                for j in range(j0, j0 + 4):
                    nc.tensor.matmul(ps_m, kv_sb[:, j, 0:P], kv_sb[:, j, P:2 * P],
                                     start=(j == 0), stop=(j == SUB - 1),
                                     skip_group_check=True)

            for j in range(4, SUB):
                kv_chunk(j)
                if j % 4 == 3 and j >= 7:
                    m_group(j - 7)
            m_group(12)

            m_sb = small.tile([P, P], F16)
            nc.vector.tensor_copy(out=m_sb, in_=ps_m)

            # ---- baseT = M^T @ qT  -> [P, SH] ----
            ps_bt = psacc.tile([P, SH], F32, tag="q")
            nc.tensor.matmul(ps_bt, m_sb, qT_sb, start=True, stop=True)
            bT_sb = small.tile([P, SH], F16)
            nc.vector.tensor_copy(out=bT_sb, in_=ps_bt)

            # ---- out = baseT^T @ weff (4 token chunks x two 512-col halves) ----
            for i in range(SH // 128):
                o_sb = outp.tile([128, E], F32, tag="o")
                for h in range(2):
                    idx = 2 * i + h
                    if idx % 2 == 0:
                        ps = pso.tile([128, 512], F32, tag="po")
                    else:
                        ps = pskv.tile([128, 512], F32, tag="kv")
                    nc.tensor.matmul(ps, bT_sb[:, i * 128:(i + 1) * 128],
                                     weff_sb[:, h * 512:(h + 1) * 512],
                                     start=True, stop=True)
                    if (i + h) % 2 == 0:
                        nc.vector.tensor_copy(out=o_sb[:, h * 512:(h + 1) * 512], in_=ps)
                    else:
                        nc.scalar.copy(out=o_sb[:, h * 512:(h + 1) * 512], in_=ps)
                nc.sync.dma_start(out=out[i * 128:(i + 1) * 128, :], in_=o_sb)

    _fix_excess_waits(nc)
    return nc


def _host_prep(x, WQ, WK, WV, result_weight, proj_w, proj_b,
               q1_vector, k1_vector, q2_vector, k2_vector, lambda_init):
    f64 = np.float64
    scale = 1.0 / math.sqrt(E // H)
    lam = (math.exp(float(np.dot(q1_vector.astype(f64), k1_vector.astype(f64))))
           - math.exp(float(np.dot(q2_vector.astype(f64), k2_vector.astype(f64))))
           + float(lambda_init[0]))

    wq_eff = WQ.astype(f64) @ proj_w.astype(f64)   # [E, P]
    wk_eff = WK.astype(f64) @ proj_w.astype(f64)
    wv_eff = WV.astype(f64) @ proj_w.astype(f64)

    d = np.concatenate([np.full(P // 2, scale), np.full(P // 2, -scale * lam)])
    wq_s = wq_eff * d
    bq_s = proj_b.astype(f64) * d

    mult = np.arange(1, H + 1, dtype=f64)
    weff = (result_weight.astype(f64).reshape(H, P, E) * mult[:, None, None]).sum(0)  # [P, E]

    wkvq = np.concatenate([wk_eff, wv_eff, wq_s], axis=1)          # [E, 3P]
    wkvq16 = wkvq.astype(np.float16).reshape(KO, 128, 3 * P).transpose(1, 0, 2)

    rows = np.zeros((448,), np.float16)
    rows[0:P] = proj_b.astype(np.float16)
    rows[P:2 * P] = proj_b.astype(np.float16)
    rows[192:448] = 1.0
    wcomb16 = np.zeros((128, KO * 3 * P + 448), np.float16)
    wcomb16[:, 0:KO * 3 * P] = wkvq16.reshape(128, KO * 3 * P)
    wcomb16[0, KO * 3 * P:] = rows
    weff16 = np.concatenate([weff, bq_s[:, None]], axis=1).astype(np.float16)  # [P, E+1]

    in_maps = []
    for c in range(N_CORES):
        b = c // (N_CORES // B)
        s0 = (c % (N_CORES // B)) * SH
        xT = x[b].T                                    # [E, S] f32 view
        xrot = np.concatenate([xT[:, s0:], xT[:, :s0]], axis=1) if s0 else xT
        # [ki, CH, KO, TPC]: e = ko*128 + ki, t = i*TPC + tt
        xt16 = (xrot.astype(np.float16)
                .reshape(KO, 128, CH, TPC)     # [ko, ki, i, tt]
                .transpose(1, 2, 0, 3))        # [ki, i, ko, tt]
        in_maps.append({
            "xt": np.ascontiguousarray(xt16),
            "wcomb": wcomb16,
            "weff": np.ascontiguousarray(weff16),
        })
    return in_maps


_NC_CACHE = {}


def kernel(**inputs):
    inputs = {k: np.asarray(v) for k, v in inputs.items()}
    in_maps = _host_prep(**inputs)
    if "nc" not in _NC_CACHE:
        _NC_CACHE["nc"] = build_bass()
    res = run_bass_kernel_spmd(_NC_CACHE["nc"], in_maps, list(range(N_CORES)))
    out = np.empty((B, S, E), np.float32)
    for c in range(N_CORES):
        b = c // (N_CORES // B)
        s0 = (c % (N_CORES // B)) * SH
        out[b, s0:s0 + SH] = res.results[c]["out"]
    return out


# revision 25
# speedup vs baseline: 1.0952x; 1.0952x over previous
"""Trainium2 Bass kernel for nn_MultiLatentAttention (B=2, S=2048, E=1024, H=16, P=64).

Math (exact reassociation of the reference):
  q = (x@WQ)@proj_w + proj_b          ->  x @ (WQ@proj_w) + proj_b
  attn1 - lam*attn2                   ->  q' @ k^T with q' = [s*q1, -s*lam*q2]
  (q'k^T) v                           ->  q' @ (k^T v)      (linear attention, no softmax)
  heads @ result_weight               ->  base @ W_eff,  W_eff[p,e] = sum_h (h+1)*RW[h*64+p, e]

Sharding: 8 cores, token-parallel for q/base/out (512 tokens each).  k^T v needs a
full-batch reduction; collectives cost 50-80us on this runtime, so each core instead
computes k,v over its ENTIRE batch (x^T for the full batch is staged per-core, fp16,
with columns rotated so the core's own q-tokens are columns 0:512 -- k^T v is
permutation-invariant over tokens).
"""

import math

import numpy as np

import concourse.bass as bass
import concourse.tile as tile
from concourse import mybir
from concourse.bass_utils import run_bass_kernel_spmd

E = 1024
H = 16
P = 64        # per-head width (latent/H)
B = 2
S = 2048
N_CORES = 8
SH = 512      # q-tokens per core
KO = E // 128    # 8 contraction chunks
CH = 8           # xt DMA chunks (256 tokens each)
TPC = S // CH    # 256 tokens per DMA chunk
SUB = S // 128   # 16 compute sub-chunks of 128 tokens

F16 = mybir.dt.float16
F32 = mybir.dt.float32


def _fix_excess_waits(nc, keep=1):
    """Split instructions with >keep sem waits (this walrus rejects multi-wait Drains)."""
    n_fixed = 0
    for f in nc.m.functions:
        for bb in f.blocks:
            insts = bb.instructions
            i = 0
            while i < len(insts):
                inst = insts[i]
                si = inst.sync_info
                waits = list(si.on_wait) if si is not None else []
                if len(waits) > keep:
                    excess, kept = waits[:-keep], waits[-keep:]
                    inst.sync_info = mybir.SyncInfo(on_wait=kept, on_update=list(si.on_update))
                    for k, w in enumerate(excess):
                        ev = mybir.InstEventSemaphore(
                            name=nc.get_next_instruction_name(),
                            engine=inst.engine, ins=[], outs=[],
                            sync_info=mybir.SyncInfo(on_wait=[w], on_update=[]),
                        )
                        nc.register_instruction(ev)
                        insts.insert(i + k, ev)
                    i += len(excess)
                    n_fixed += 1
                i += 1
    return n_fixed


def build_bass():
    nc = bass.Bass(num_devices=N_CORES, enable_partition_id=False)
    # xt: [128(ki), CH, KO, TPC] -- per-partition contiguous per chunk
    xt = nc.declare_dram_parameter("xt", [128, CH, KO, TPC], F16, isOutput=False)
    # wcomb: [wkvq (KO*192) | rows(448, partition 0 only)] per partition
    WCOLS = KO * 3 * P + 448
    wcomb = nc.declare_dram_parameter("wcomb", [128, WCOLS], F16, isOutput=False)
    weff = nc.declare_dram_parameter("weff", [P, E + 1], F16, isOutput=False)
    out = nc.declare_dram_parameter("out", [SH, E], F32, isOutput=True)

    with tile.TileContext(nc) as tc:
        with (
            tc.tile_pool(name="singles", bufs=1) as singles,
            tc.tile_pool(name="xtp", bufs=CH) as xtp,
            tc.tile_pool(name="kvp", bufs=1) as kvp,
            tc.tile_pool(name="small", bufs=1) as small,
            tc.tile_pool(name="outp", bufs=3) as outp,
            tc.tile_pool(name="pskv", bufs=4, space="PSUM") as pskv,
            tc.tile_pool(name="psacc", bufs=1, space="PSUM") as psacc,
            tc.tile_pool(name="pso", bufs=3, space="PSUM") as pso,
        ):
            # ---- first xt chunk, then weights, then the rest of xt ----
            xt_tiles = [None] * CH

            def load_chunk(i, split=False):
                t = xtp.tile([128, KO, TPC], F16, tag="xt")
                if split:
                    nc.sync.dma_start(out=t[:, :, 0:128], in_=xt[:, i, :, 0:128])
                    nc.sync.dma_start(out=t[:, :, 128:TPC], in_=xt[:, i, :, 128:TPC])
                else:
                    nc.sync.dma_start(out=t, in_=xt[:, i])
                xt_tiles[i] = t

            wc_sb = singles.tile([128, KO * 3 * P + 448], F16)
            nc.sync.dma_start(out=wc_sb[:, 0:2 * 3 * P],
                              in_=wcomb[:, 0:2 * 3 * P])
            load_chunk(0, split=True)
            nc.sync.dma_start(out=wc_sb[:, 2 * 3 * P:],
                              in_=wcomb[:, 2 * 3 * P:])
            wkvq_sb = wc_sb[:, 0:KO * 3 * P].rearrange("p (ko c) -> p ko c", ko=KO)
            rows_sb = wc_sb[0:1, KO * 3 * P:]
            bkv_sb = rows_sb[:, 0:128]
            ones_sb = rows_sb[:, 192:448]
            for i in range(1, CH):
                load_chunk(i)
            weff_sb = singles.tile([P, E + 1], F16)
            nc.sync.dma_start(out=weff_sb, in_=weff[:, :])

            kv_sb = kvp.tile([128, SUB, 2 * P], F16)

            # ---- k|v for every token sub-chunk (dense PE stream) ----
            def kv_chunk(j):
                i, half = j // 2, (j % 2) * 128
                ps = pskv.tile([128, 2 * P], F32, tag="kv")
                for ko in range(KO):
                    nc.tensor.matmul(ps, xt_tiles[i][:, ko, half:half + 128],
                                     wkvq_sb[:, ko, 0:2 * P],
                                     start=(ko == 0), stop=False)
                nc.tensor.matmul(ps, ones_sb[:, 0:128], bkv_sb, start=False, stop=True)
                if j % 2 == 0:
                    nc.vector.tensor_copy(out=kv_sb[:, j], in_=ps)
                else:
                    nc.scalar.copy(out=kv_sb[:, j], in_=ps)

            for j in range(4):
                kv_chunk(j)

            # ---- qT = wq^T @ xt[:, 0:512] + bq  -> [P, SH] (ko-outer, N=256) ----
            ps_q = psacc.tile([P, SH], F32, tag="acc", name="ps_q")
            for i in range(2):
                for ko in range(KO):
                    nc.tensor.matmul(ps_q[:, i * TPC:(i + 1) * TPC],
                                     wkvq_sb[:, ko, 2 * P:3 * P],
                                     xt_tiles[i][:, ko],
                                     start=(ko == 0), stop=(ko == KO - 1),
                                     skip_group_check=True)
            qT_sb = small.tile([P, SH], F16)
            nc.scalar.activation(out=qT_sb, in_=ps_q,
                                 func=mybir.ActivationFunctionType.Identity,
                                 bias=weff_sb[:, E:E + 1])

            # ---- M = k^T v, accumulated in groups of 4 sub-chunks ----
            ps_m_full = psacc.tile([P, SH], F32, tag="acc", name="ps_m")
            ps_m = ps_m_full[:, 0:P]

            def m_group(j0):
                for j in range(j0, j0 + 4):
                    nc.tensor.matmul(ps_m, kv_sb[:, j, 0:P], kv_sb[:, j, P:2 * P],
                                     start=(j == 0), stop=(j == SUB - 1),
                                     skip_group_check=True)

            for j in range(4, SUB):
                kv_chunk(j)
                if j % 4 == 3 and j >= 7:
                    m_group(j - 7)
            m_group(12)

            m_sb = small.tile([P, P], F16)
            nc.vector.tensor_copy(out=m_sb, in_=ps_m)

            # ---- baseT = M^T @ qT  -> [P, SH] ----
            ps_bt = psacc.tile([P, SH], F32, tag="acc", name="ps_bt")
            nc.tensor.matmul(ps_bt, m_sb, qT_sb, start=True, stop=True)
            bT_sb = small.tile([P, SH], F16)
            nc.vector.tensor_copy(out=bT_sb, in_=ps_bt)

            # ---- out = baseT^T @ weff (4 token chunks x two 512-col halves) ----
            for i in range(SH // 128):
                o_sb = outp.tile([128, E], F32, tag="o")
                for h in range(2):
                    idx = 2 * i + h
                    if idx % 2 == 0:
                        ps = pso.tile([128, 512], F32, tag="po")
                    else:
                        ps = pskv.tile([128, 512], F32, tag="kv")
                    nc.tensor.matmul(ps, bT_sb[:, i * 128:(i + 1) * 128],
                                     weff_sb[:, h * 512:(h + 1) * 512],
                                     start=True, stop=True)
                    if (i + h) % 2 == 0:
                        nc.vector.tensor_copy(out=o_sb[:, h * 512:(h + 1) * 512], in_=ps)
                    else:
                        nc.scalar.copy(out=o_sb[:, h * 512:(h + 1) * 512], in_=ps)
                nc.sync.dma_start(out=out[i * 128:(i + 1) * 128, :], in_=o_sb)

    _fix_excess_waits(nc)
    return nc


def _host_prep(x, WQ, WK, WV, result_weight, proj_w, proj_b,
               q1_vector, k1_vector, q2_vector, k2_vector, lambda_init):
    f64 = np.float64
    scale = 1.0 / math.sqrt(E // H)
    lam = (math.exp(float(np.dot(q1_vector.astype(f64), k1_vector.astype(f64))))
           - math.exp(float(np.dot(q2_vector.astype(f64), k2_vector.astype(f64))))
           + float(lambda_init[0]))

    wq_eff = WQ @ proj_w   # [E, P] f32
    wk_eff = WK @ proj_w
    wv_eff = WV @ proj_w

    d = np.concatenate([np.full(P // 2, scale), np.full(P // 2, -scale * lam)]).astype(np.float32)
    wq_s = wq_eff * d
    bq_s = proj_b * d

    mult = np.arange(1, H + 1, dtype=np.float32)
    weff = (result_weight.reshape(H, P, E) * mult[:, None, None]).sum(0, dtype=f64)  # [P, E]

    wkvq = np.concatenate([wk_eff, wv_eff, wq_s], axis=1)          # [E, 3P]
    wkvq16 = wkvq.astype(np.float16).reshape(KO, 128, 3 * P).transpose(1, 0, 2)

    rows = np.zeros((448,), np.float16)
    rows[0:P] = proj_b.astype(np.float16)
    rows[P:2 * P] = proj_b.astype(np.float16)
    rows[192:448] = 1.0
    wcomb16 = np.zeros((128, KO * 3 * P + 448), np.float16)
    wcomb16[:, 0:KO * 3 * P] = wkvq16.reshape(128, KO * 3 * P)
    wcomb16[0, KO * 3 * P:] = rows
    weff16 = np.concatenate([weff, bq_s[:, None]], axis=1).astype(np.float16)  # [P, E+1]

    in_maps = []
    xt_cache = {}
    for c in range(N_CORES):
        b = c // (N_CORES // B)
        s0 = (c % (N_CORES // B)) * SH
        xT = x[b].T                                    # [E, S] f32 view
        xrot = np.concatenate([xT[:, s0:], xT[:, :s0]], axis=1) if s0 else xT
        # [ki, CH, KO, TPC]: e = ko*128 + ki, t = i*TPC + tt
        xt16 = (xrot.astype(np.float16)
                .reshape(KO, 128, CH, TPC)     # [ko, ki, i, tt]
                .transpose(1, 2, 0, 3))        # [ki, i, ko, tt]
        in_maps.append({
            "xt": np.ascontiguousarray(xt16),
            "wcomb": wcomb16,
            "weff": np.ascontiguousarray(weff16),
        })
    return in_maps


_NC_CACHE = {}


def kernel(**inputs):
    inputs = {k: np.asarray(v) for k, v in inputs.items()}
    in_maps = _host_prep(**inputs)
    if "nc" not in _NC_CACHE:
        _NC_CACHE["nc"] = build_bass()
    res = run_bass_kernel_spmd(_NC_CACHE["nc"], in_maps, list(range(N_CORES)))
    out = np.empty((B, S, E), np.float32)
    for c in range(N_CORES):
        b = c // (N_CORES // B)
        s0 = (c % (N_CORES // B)) * SH
        out[b, s0:s0 + SH] = res.results[c]["out"]
    return out
